# revision 38
# baseline (speedup 1.0000x reference)
"""GAT-style node-feature kernel for Trainium2 (8 NeuronCores, SPMD).

Problem: y = Linear_o(MHA(x) with per-edge gate mask), where the mask is
mean(edge_gate, axis=-1) (B,V,V) applied multiplicatively to attention
scores before softmax.  edge_gate is (2,768,768,128) fp32 = 604 MB; its
HBM read dominates (memory-bound regime).

Sharding: the 1536 (batch*query) rows are split into 8 chunks of 192
(cores 0-3 -> batch 0, cores 4-7 -> batch 1).  Each core reads only its
75.5 MB slice of edge_gate, computes the channel-sum on the vector
engine, and runs its queries' attention against the full 768 keys of its
batch (k/v replicated per batch).  Host-side prep transposes the small
weights/x so the kernel needs no on-chip layout changes.

Key layout trick: scores are computed TRANSPOSED (keys on partitions,
queries on free dim).  The channel-reduce of edge_gate then lands
directly in the layout the mask-multiply needs (no transposes), softmax
denominators come from an appended ones-column in the value matrix, and
exp() needs no max-subtraction (|scores*mask| < ~10, fp32-safe).

Keys are processed in a permuted order (k' = r*128+p  <->  k = 6p+r, from
the contiguous-DMA-friendly reduce tiling); softmax and the attended sum
are permutation-invariant, so only the host-side column order of x^T
changes.
"""

import numpy as np
from contextlib import ExitStack

P = 128          # partitions / hidden dim
HID = 128
NH = 8
HD = 16
V = 768          # keys per batch
NQ = 192         # queries per core
R = 6            # rows-per-partition in the reduce tiling (768 = 128*6)
# (start, size) of each DMA/reduce group; chunk 0 = q[0:128), chunk 1 =
# q[128:192).  The last groups are small so the final reduce (which sits
# on the critical tail) is short.
import os as _os
_GQ_CFG = _os.environ.get("KERNEL_GQ", "8tail")
if _GQ_CFG == "16":
    GQ = 16
    GROUPS = [(i * 16, 16) for i in range(8)] + \
             [(128, 16), (144, 16), (160, 16), (176, 8), (184, 8)]
elif _GQ_CFG == "12":
    GQ = 12
    GROUPS = [(i * 12, 12) for i in range(10)] + [(120, 8)] + \
             [(128 + i * 12, 12) for i in range(5)] + [(188, 4)]
elif _GQ_CFG == "8tail":
    GQ = 8
    GROUPS = [(i * 8, 8) for i in range(23)] + [(184, 4), (188, 2), (190, 2)]
else:
    GQ = 8
    GROUPS = [(i * 8, 8) for i in range(24)]
EG_BUFS = int(_os.environ.get("KERNEL_EG_BUFS", "4"))
# diagnostic modes: "full" (default), "dmaonly" (just the edge_gate
# stream), "dmared" (stream + reduces, no attention)
KMODE = _os.environ.get("KERNEL_MODE", "full")
# attention chunks (start, size): the last ones are small because the
# final chunk's attention is the post-DMA tail
if _os.environ.get("KERNEL_CHUNKS", "64x3") == "tail32":
    CHUNKS = [(0, 64), (64, 64), (128, 32), (160, 32)]
else:
    CHUNKS = [(0, 64), (64, 64), (128, 64)]
N_CORES = 8
QSCALE = 1.0 / 512.0   # 1/sqrt(hd) * 1/channels = 1/4 * 1/128

_cached = {}


def _build_module(repeat=1):
    """Build the per-core Bass module.

    repeat > 1 wraps the whole body in a hardware For_i loop re-running it
    on identical inputs -- used only for timing (amortizes host dispatch).
    """
    import concourse.bass as bass
    import concourse.tile as tile
    from concourse import bacc, mybir
    from concourse.masks import make_identity
    from contextlib import nullcontext

    f32 = mybir.dt.float32
    AFT = mybir.ActivationFunctionType
    AX = mybir.AxisListType

    nc = bacc.Bacc("TRN2", target_bir_lowering=False, debug=False)

    eg = nc.dram_tensor("eg", [NQ, V, HID], f32, kind="ExternalInput").ap()
    xqT = nc.dram_tensor("xqT", [P, NQ], f32, kind="ExternalInput").ap()
    xkT = nc.dram_tensor("xkT", [P, V], f32, kind="ExternalInput").ap()
    wqT = nc.dram_tensor("wqT", [P, P], f32, kind="ExternalInput").ap()
    wkT = nc.dram_tensor("wkT", [P, P], f32, kind="ExternalInput").ap()
    wvT = nc.dram_tensor("wvT", [P, P], f32, kind="ExternalInput").ap()
    woT = nc.dram_tensor("woT", [P, P], f32, kind="ExternalInput").ap()
    bqs = nc.dram_tensor("bqs", [HD, NH], f32, kind="ExternalInput").ap()
    bkc = nc.dram_tensor("bkc", [HD, NH], f32, kind="ExternalInput").ap()
    bvr = nc.dram_tensor("bvr", [1, P], f32, kind="ExternalInput").ap()
    bor = nc.dram_tensor("bor", [1, P], f32, kind="ExternalInput").ap()
    out = nc.dram_tensor("out", [NQ, HID], f32, kind="ExternalOutput").ap()

    # edge_gate viewed so partition p holds rows 6p..6p+5 of each query's
    # (768,128) block: 3 KB contiguous per partition per query.
    eg_r = eg.rearrange("q (p r) c -> p q r c", p=P)

    with tile.TileContext(nc) as tc, ExitStack() as ctx:
        singles = ctx.enter_context(tc.tile_pool(name="singles", bufs=1))
        egp = ctx.enter_context(tc.tile_pool(name="egp", bufs=EG_BUFS))
        workp = ctx.enter_context(tc.tile_pool(name="work", bufs=2))
        wexpp = ctx.enter_context(tc.tile_pool(name="wexp", bufs=2))
        pp_sc = ctx.enter_context(tc.tile_pool(name="psc", bufs=4, space="PSUM"))
        pp_acc = ctx.enter_context(tc.tile_pool(name="pacc", bufs=2, space="PSUM"))
        pp_misc = ctx.enter_context(tc.tile_pool(name="pmisc", bufs=2, space="PSUM"))

        if repeat == 1:
            loop_cm = nullcontext()
        else:
            ET = mybir.EngineType
            loop_cm = tc.For_i(0, repeat, 1,
                               hint_engines=(ET.PE, ET.DVE, ET.Activation,
                                             ET.SP, ET.Pool))
        ctx.enter_context(loop_cm)

        # ---- constants / small inputs ----
        wqT_t = singles.tile([P, P], f32)
        nc.scalar.dma_start(wqT_t[:], wqT)
        wkT_t = singles.tile([P, P], f32)
        nc.scalar.dma_start(wkT_t[:], wkT)
        wvT_t = singles.tile([P, P], f32)
        nc.scalar.dma_start(wvT_t[:], wvT)
        woT_t = singles.tile([P, P], f32)
        nc.scalar.dma_start(woT_t[:], woT)
        xqT_t = singles.tile([P, NQ], f32)
        nc.scalar.dma_start(xqT_t[:], xqT)
        xkT_t = singles.tile([P, V], f32)
        nc.scalar.dma_start(xkT_t[:], xkT)
        bqs_t = singles.tile([HD, NH], f32)
        nc.scalar.dma_start(bqs_t[:], bqs)
        bkc_t = singles.tile([HD, NH], f32)
        nc.scalar.dma_start(bkc_t[:], bkc)
        bvr_t = singles.tile([1, P], f32)
        nc.scalar.dma_start(bvr_t[:], bvr)
        bor_t = singles.tile([1, P], f32)
        nc.scalar.dma_start(bor_t[:], bor)

        ones_t = singles.tile([1, P], f32)
        nc.vector.memset(ones_t[:], 1.0)
        ident = singles.tile([P, P], f32)
        make_identity(nc, ident[:])

        # channel-sums of edge_gate: mbuf[p, t, r] = sum_c eg[t, 6p+r, c],
        # one tile per attention chunk
        mbufs = [singles.tile([P, csz, R], f32, name=f"mbuf{i}", tag=f"mbuf{i}")
                 for i, (_, csz) in enumerate(CHUNKS)]

        # head-major layouts (PE operands must start at partition 0)
        qT_t = singles.tile([HD, NH, NQ], f32)   # (d, head, query), scaled 1/512
        kT_t = singles.tile([HD, NH, V], f32)    # (d, head, key') permuted keys
        # v matrix with a ones-column appended per head: (k', head, 17)
        v_aug = singles.tile([P, R, NH, HD + 1], f32)

        # ---- q/k/v projections ----
        # per-head matmuls: engine reads must start at 32-aligned partitions,
        # so (16, ...) operands live at partition base 0 and heads are
        # separated via free-dim slices of the transposed weights.
        for h in range(NH):
            qps = pp_misc.tile([HD, NQ], f32, tag="misc")
            nc.tensor.matmul(qps[:], wqT_t[:, h * HD:(h + 1) * HD], xqT_t[:],
                             start=True, stop=True)
            nc.scalar.activation(qT_t[:, h, :], qps[:], AFT.Identity,
                                 bias=bqs_t[:, h:h + 1], scale=QSCALE)
            for half in range(2):
                kps = pp_misc.tile([HD, 384], f32, tag="misc")
                nc.tensor.matmul(kps[:], wkT_t[:, h * HD:(h + 1) * HD],
                                 xkT_t[:, 384 * half:384 * (half + 1)],
                                 start=True, stop=True)
                nc.scalar.activation(kT_t[:, h, 384 * half:384 * (half + 1)],
                                     kps[:], AFT.Identity,
                                     bias=bkc_t[:, h:h + 1], scale=1.0)

        nc.vector.memset(v_aug[:], 1.0)   # ones-columns survive the copies below
        for j in range(R):
            vps = pp_misc.tile([P, P], f32, tag="misc")
            nc.tensor.matmul(vps[:], ones_t[:], bvr_t[:], start=True, stop=False)
            nc.tensor.matmul(vps[:], xkT_t[:, j * P:(j + 1) * P], wvT_t[:],
                             start=False, stop=True)
            nc.scalar.copy(v_aug[:, j, :, 0:HD],
                           vps[:].rearrange("p (h d) -> p h d", h=NH))

        # ---- attention for one chunk of queries ----
        # Emitted with a large priority offset (when enabled) so the Tile
        # scheduler prefers the DMA+reduce stream whenever both are ready;
        # attention then fills engine gaps instead of stalling the stream.
        atn_lowpri = _os.environ.get("KERNEL_ATN_LOWPRI", "1") == "1"

        def attention(ci):
            saved_pri = tc.cur_priority
            if atn_lowpri:
                tc.cur_priority = 1_000_000 + ci * 10_000
            t0, tcq = CHUNKS[ci]
            mb = mbufs[ci]
            aug = pp_acc.tile([tcq, NH, HD + 1], f32)
            # mask viewed as (p, block, query) to match the scores layout
            mbT = mb[:, :, :].rearrange("p t r -> p r t")
            for h in range(NH):
                # one PSUM tile holds all 6 key-blocks' transposed scores so
                # the mask-multiply and exp are single fat ops (the per-block
                # version was latency-bound on PE->DVE->ACT->PE hops)
                sc = pp_sc.tile([P, R, tcq], f32, tag="sc")
                for j in range(R):
                    nc.tensor.matmul(
                        sc[:, j, :],
                        kT_t[:, h, j * P:(j + 1) * P],
                        qT_t[:, h, t0:t0 + tcq],
                        start=True, stop=True)
                nc.vector.tensor_mul(sc[:], sc[:], mbT)
                wexp = wexpp.tile([P, R, tcq], f32, tag="wexp")
                nc.scalar.activation(wexp[:], sc[:], AFT.Exp)
                for j in range(R):
                    nc.tensor.matmul(aug[:, h, :], wexp[:, j, :],
                                     v_aug[:, j, h, :],
                                     start=(j == 0), stop=(j == R - 1))
            recip = workp.tile([tcq, NH], f32)
            nc.vector.reciprocal(recip[:], aug[:, :, HD])
            att_sb = workp.tile([tcq, P], f32)
            for h in range(NH):
                nc.vector.tensor_scalar_mul(att_sb[:, h * HD:(h + 1) * HD],
                                            aug[:, h, 0:HD], recip[:, h:h + 1])
            attT_ps = pp_misc.tile([P, 128], f32, tag="misc")
            nc.tensor.transpose(attT_ps[:, 0:tcq], att_sb[:], ident[0:tcq, 0:tcq])
            attT_sb = workp.tile([P, 128], f32)
            nc.scalar.copy(attT_sb[:, 0:tcq], attT_ps[:, 0:tcq])
            yps = pp_misc.tile([128, P], f32, tag="misc")
            nc.tensor.matmul(yps[0:tcq, :], ones_t[0:1, 0:tcq], bor_t[:],
                             start=True, stop=False)
            nc.tensor.matmul(yps[0:tcq, :], attT_sb[:, 0:tcq], woT_t[:],
                             start=False, stop=True)
            y_sb = workp.tile([128, P], f32)
            nc.vector.tensor_copy(y_sb[0:tcq, :], yps[0:tcq, :])
            nc.scalar.dma_start(out[t0:t0 + tcq, :], y_sb[0:tcq, :])
            if atn_lowpri:
                tc.cur_priority = saved_pri

        # ---- main stream: DMA edge_gate slices + channel-sum reduce ----
        alt_rings = _os.environ.get("KERNEL_ALT_RINGS", "0") == "1"
        for gi, (q0, gq) in enumerate(GROUPS):
            egt = egp.tile([P, GQ, R, HID], f32, tag="eg")
            eng = nc.scalar if (alt_rings and gi % 2) else nc.sync
            eng.dma_start(egt[:, 0:gq, :, :], eg_r[:, q0:q0 + gq, :, :])
            if KMODE == "dmaonly":
                continue
            ci = max(i for i, (c0, _) in enumerate(CHUNKS) if c0 <= q0)
            c0 = CHUNKS[ci][0]
            mb_out = mbufs[ci][:, q0 - c0:q0 - c0 + gq, :]
            nc.vector.reduce_sum(mb_out, egt[:, 0:gq, :, :], axis=AX.X)
            if KMODE == "full" and any(q0 + gq == c0 + csz
                                       for (c0, csz) in CHUNKS):
                attention(next(i for i, (c0, csz) in enumerate(CHUNKS)
                               if q0 + gq == c0 + csz))
        if KMODE != "full":
            ytmp = workp.tile([P, HID], f32)
            nc.vector.memset(ytmp[:], 0.0)
            nc.scalar.dma_start(out[0:128, :], ytmp[:])
            nc.scalar.dma_start(out[128:192, :], ytmp[0:64, :])

    nc.compile()
    return nc


def _get_module(repeat=1):
    if repeat not in _cached:
        _cached[repeat] = _build_module(repeat)
    return _cached[repeat]


def kernel(x, edge_gate, Wq, bq, Wk, bk, Wv, bv, Wo, bo):
    from concourse.bass_utils import run_bass_kernel_spmd

    x = np.asarray(x, dtype=np.float32)
    edge_gate = np.asarray(edge_gate, dtype=np.float32)
    B, Vv, H = x.shape

    # permuted key order: column j of xkT is original key 6*(j%128) + j//128
    jj = np.arange(V)
    perm = 6 * (jj % P) + jj // P

    common = {
        "wqT": np.ascontiguousarray(np.asarray(Wq, np.float32).T),
        "wkT": np.ascontiguousarray(np.asarray(Wk, np.float32).T),
        "wvT": np.ascontiguousarray(np.asarray(Wv, np.float32).T),
        "woT": np.ascontiguousarray(np.asarray(Wo, np.float32).T),
        "bqs": np.ascontiguousarray((np.asarray(bq, np.float32) * QSCALE)
                                    .reshape(NH, HD).T),
        "bkc": np.ascontiguousarray(np.asarray(bk, np.float32).reshape(NH, HD).T),
        "bvr": np.ascontiguousarray(np.asarray(bv, np.float32).reshape(1, P)),
        "bor": np.ascontiguousarray(np.asarray(bo, np.float32).reshape(1, P)),
    }

    in_maps = []
    for c in range(N_CORES):
        b = c // 4
        q0 = (c % 4) * NQ
        xb = x[b]
        m = dict(common)
        m["eg"] = np.ascontiguousarray(edge_gate[b, q0:q0 + NQ])
        m["xqT"] = np.ascontiguousarray(xb[q0:q0 + NQ].T)
        m["xkT"] = np.ascontiguousarray(xb[perm].T)
        in_maps.append(m)

    nc = _get_module()
    res = run_bass_kernel_spmd(nc, in_maps, core_ids=list(range(N_CORES)))
    y = np.stack([r["out"] for r in res.results], axis=0)  # (8, 192, 128)
    return y.reshape(B, Vv, H)
